# revision 43
# baseline (speedup 1.0000x reference)
# Trainium2 Bass kernel for nn_AxonalConnections (gnn_message_passing).
#
# Computes out[B, H, W] = (spikes.reshape(B, N) @ adjacency.T).reshape(B, H, W)
# with B=16, H=W=128, N=16384 on 8 NeuronCores.
#
# Strategy (pure tensor parallelism, no collectives):
#   - Shard adjacency row-wise (target dim) across 8 cores: core i owns
#     target columns [i*2048, (i+1)*2048) of the output.
#   - The kernel is HBM/DMA-bandwidth bound, so minimize shipped bytes:
#     * input-adaptive block pruning: the host scans the adjacency at
#       [128 x 128] block granularity (source grid-row si x target grid-row
#       ti) and ships only blocks that contain nonzeros. For the conv-
#       structured adjacency this is ~112 of 2048 blocks per core (1.75 MiB
#       vs 64 MiB fp8); for dense inputs every block ships and the kernel
#       stays exact up to quantization. Per-core block sets are aligned by
#       a per-core source offset into one shared pattern so all 8 cores run
#       the same NEFF.
#     * fp8e3 (e3m4) adjacency stream with a per-core scale chosen so the
#       largest magnitude lands at ~8.0: measured 8.7e-3 relative output
#       error on the conv-structured data — 2.3x inside the 2e-2 accuracy
#       budget — at half the bytes of fp16. Spikes stay fp16 (stationary,
#       tiny). The host divides the scale back out of the output.
#   - Blocks stream si-major with merged matmuls over consecutive ti; each
#     PSUM bank finishes early and its PSUM->SBUF copy + output DMA overlap
#     the remaining matmuls. The final bank's store is split across both
#     HWDGE rings so its halves move concurrently. Output is bank-major in
#     DRAM so stores coalesce; the host de-interleaves.
#   - A PE warmup (24 x N=128 matmuls on a zeroed tile) spans the HAM cold
#     window while the first DMA groups arrive and bridges into the real
#     matmuls with no PE-idle gap, so they run at 2.4 GHz almost
#     immediately.
#   - DMA groups are aligned to si-run boundaries (a group edge never
#     splits a matmul segment). Head groups are small (4..16 blocks) so the
#     parade of per-group completion receipts starts the PE early; middle
#     groups of ~24 blocks amortize receipt overhead; a small tail group
#     keeps the after-last-byte matmul residue short.

import numpy as np

B = 16
H = 128
W = 128
N = H * W            # 16384 source == target size
NCORES = 8
TSH = N // NCORES    # 2048 target columns per core
TI = TSH // W        # 16 target grid-rows per core
P = 128              # SBUF partitions / contraction tile
SCHUNKS = N // P     # 128 source chunks (== source grid-rows)
BLK = P * P          # elements per block
FP8_MAX = 8.0        # target magnitude for the fp8e3 (e3m4) quantization
N_WARM = 24          # PE warmup matmuls (N=128 each; must bridge into the
                     # real matmuls with no PE-idle gap or HAM re-throttles)

_cache = {}


def _group_sizes(pattern):
    """DMA group sizes (blocks per dma_start), aligned to si-run boundaries.

    A matmul segment never spans two DMA groups, so aligning group edges to
    si-run edges avoids splitting segments (each split costs an extra
    LDWEIGHTS + matmul dispatch on the PE). Head groups are small so the
    parade of per-group completion receipts starts the PE early; a small
    tail group keeps the after-last-byte matmul residue short."""
    run_sizes = []
    for _, s in pattern:
        if run_sizes and s == run_sizes[-1][0]:
            run_sizes[-1][1] += 1
        else:
            run_sizes.append([s, 1])
    runs = [r[1] for r in run_sizes]
    targets = [4, 8, 12, 16] + [24] * 100
    sizes = []
    cur = 0
    ti_idx = 0
    total = sum(runs)
    done = 0
    for r in runs:
        cur += r
        done += r
        rem = total - done
        tgt = targets[min(ti_idx, len(targets) - 1)]
        # close the group when it reaches its target, but keep the final
        # group small (it sets the post-stream matmul residue).
        if cur >= tgt and (rem == 0 or rem >= 4):
            sizes.append(cur)
            cur = 0
            ti_idx += 1
    if cur:
        sizes.append(cur)
    return sizes


def _plan_segments(pattern, group_sizes):
    """Plan merged matmuls over the si-major block stream.

    pattern: list of (ti, si_rel), si-major then ti-ascending — the stream
    order. Blocks with consecutive ti, the same source chunk, the same PSUM
    bank, and the same DMA group merge into one matmul of N = 128*len.

    start=True is set ONLY on the first segment of each PSUM bank: on HW it
    clears has_written for the WHOLE bank, and the per-element has_written
    bit then makes every region's first write an overwrite and later writes
    accumulate.

    Returns segments: list of (k0, nblk, si_rel, ti0, start).
    """
    group_of = []
    for g, gsz in enumerate(group_sizes):
        group_of += [g] * gsz
    segments = []
    k = 0
    n = len(pattern)
    seen_banks = set()
    while k < n:
        ti0, s = pattern[k]
        ln = 1
        while (
            k + ln < n
            and pattern[k + ln] == (ti0 + ln, s)
            and (ti0 + ln) // 4 == ti0 // 4
            and group_of[k + ln] == group_of[k]
        ):
            ln += 1
        bank = ti0 // 4
        segments.append((k, ln, s, ti0, bank not in seen_banks))
        seen_banks.add(bank)
        k += ln
    return segments


def _build_nc(pattern, n_spk):
    """Build + compile the SPMD Bass program.

    pattern: list of (ti, si_rel) block coordinates in si-major stream
             order, identical for all cores. Every ti in [0, TI) appears.
    n_spk:   number of stationary source chunks shipped (max si_rel + 1).
    """
    import concourse.mybir as mybir
    import concourse.tile as tile
    from concourse import bacc

    n_blocks = len(pattern)
    g_sizes = _group_sizes(pattern)
    segs = _plan_segments(pattern, g_sizes)

    nc = bacc.Bacc(
        "TRN2",
        target_bir_lowering=False,
        debug=False,
        num_devices=NCORES,
        enable_partition_id=False,
    )
    # ablk: flat stream of gathered [128 x 128] fp8e3 adjacency blocks in
    # `pattern` order, packed per DMA group as [p, group_blocks*128]
    # (partition-major) so every descriptor moves one contiguous run per
    # partition.
    ablk = nc.dram_tensor(
        "ablk", [n_blocks * BLK], mybir.dt.float8e3, kind="ExternalInput"
    ).ap()
    # spk: stationary weights for the shipped source-chunk window, packed
    # [P, n_spk*B] fp16 where spk[p, k*B + b] = spikes[b, (o_i + k)*128 + p]
    # (o_i = per-core source offset; out-of-range chunks are zero).
    spk = nc.dram_tensor(
        "spk", [P, n_spk * B], mybir.dt.float16, kind="ExternalInput"
    ).ap()
    # Output is bank-major: o[j] is bank j's [B, 512] slab, contiguous in
    # DRAM so each store coalesces into large AXI bursts (a [B, TSH]-strided
    # store measured ~20 B/ns from descriptor overhead). fp16 halves the
    # drain copy (2x DVE mode) and the store bytes; the ~2e-4 rounding it
    # adds is negligible against the fp8 weight quantization. Host converts
    # back to fp32, rescales and de-interleaves.
    NJ = 4
    out = nc.dram_tensor(
        "o", [NJ, B, NJ * P], mybir.dt.float16, kind="ExternalOutput"
    ).ap()

    f32 = mybir.dt.float32
    f16 = mybir.dt.float16

    # Last stream index per PSUM bank (closes the accumulation group) and
    # the bank that closes last (its drain is the critical tail).
    last_k_bank = {}
    for k, (ti, _) in enumerate(pattern):
        last_k_bank[ti // NJ] = k
    final_bank = max(last_k_bank, key=last_k_bank.get)

    # Map stream index -> (group, local index).
    grp_of = []
    for g, gsz in enumerate(g_sizes):
        base = len(grp_of)
        grp_of += [(g, kk - base) for kk in range(base, base + gsz)]

    with tile.TileContext(nc) as tc:
        with (
            tc.tile_pool(name="adj", bufs=len(g_sizes)) as adj_pool,
            tc.tile_pool(name="spkp", bufs=1) as spk_pool,
            tc.tile_pool(name="warm", bufs=1) as warm_pool,
            tc.tile_pool(name="psum", bufs=1, space="PSUM") as psum_pool,
            tc.tile_pool(name="outp", bufs=1) as out_pool,
        ):
            ps = [
                psum_pool.tile([B, NJ * P], f32, name=f"ps{j}", tag=f"ps{j}")
                for j in range(NJ)
            ]

            # PE warmup: short N=128 matmuls on a zeroed tile into a scratch
            # PSUM bank, issued before any real data arrives. They keep the
            # PE busy across the HAM activity window so the real matmuls run
            # at 2.4 GHz, and they bridge into the real matmuls with no
            # PE-idle gap.
            dumt = warm_pool.tile([P, P], f16)
            nc.vector.memset(dumt[:], 0.0)
            psw = psum_pool.tile([B, P], f32, name="psw", tag="psw")
            for _ in range(N_WARM):
                nc.tensor.matmul(
                    psw[:, :],
                    dumt[:, 0:B],
                    dumt[:, :],
                    start=True,
                    stop=True,
                    skip_group_check=True,
                )

            # Stationary weights: the head chunks (what the first matmuls
            # need) go on the ACT ring immediately; the rest follows on the
            # same ring. Both stream concurrently with the first block
            # groups on the SP ring.
            spk_t = spk_pool.tile([P, n_spk * B], f16)
            n_head = min(4, n_spk)
            nc.scalar.dma_start(
                spk_t[:, : n_head * B], spk[:, : n_head * B]
            )
            if n_spk > n_head:
                nc.scalar.dma_start(
                    spk_t[:, n_head * B :], spk[:, n_head * B :]
                )

            ot = out_pool.tile([B, TSH], f16)

            # Head + middle groups go on the SP ring in consumption order
            # (the PE consumes groups in order, so scrambled arrival starves
            # it — measured). The TAIL ~40% of blocks go on the ACT ring:
            # they stream concurrently (behind the small spk transfers) and
            # land well before the PE reaches them, lifting aggregate input
            # bandwidth and removing the end-of-stream receipt exposure,
            # without touching the head groups the PE is chasing.
            n_blocks_total = sum(g_sizes)
            cum = 0
            split_g = len(g_sizes)
            for g, gsz in enumerate(g_sizes):
                cum += gsz
                if cum >= (n_blocks_total * 3) // 5:
                    split_g = g + 1
                    break
            at_tiles = []
            off = 0
            for g, gsz in enumerate(g_sizes):
                at = adj_pool.tile(
                    [P, gsz * P], mybir.dt.float8e3, name=f"at{g}", tag="at"
                )
                eng = nc.sync if g < split_g else nc.scalar
                eng.dma_start(
                    at[:],
                    ablk[off : off + gsz * BLK].rearrange("(p t) -> p t", p=P),
                )
                off += gsz * BLK
                at_tiles.append(at)

            for k0, nblk, si_rel, ti0, start in segs:
                g, kl = grp_of[k0]
                j, c = divmod(ti0, NJ)
                pj = ps[j]
                nc.tensor.matmul(
                    pj[:, c * P : (c + nblk) * P],
                    spk_t[:, si_rel * B : (si_rel + 1) * B],
                    at_tiles[g][:, kl * P : (kl + nblk) * P],
                    start=start,
                    stop=(k0 + nblk - 1 == last_k_bank[j]),
                    skip_group_check=True,
                )
                if k0 + nblk - 1 == last_k_bank[j]:
                    # Bank fully accumulated: drain it while the remaining
                    # banks' matmuls keep streaming. Stores go on the SP
                    # ring: all input dma_start instructions precede them in
                    # the SP program, so a store's semaphore wait can never
                    # delay an input transfer. The final bank's copy is
                    # split across Vector + Scalar engines to halve the
                    # critical-path drain.
                    sl = slice(j * NJ * P, (j + 1) * NJ * P)
                    nc.vector.tensor_copy(ot[:, sl], pj[:, :])
                    if j == final_bank:
                        # Split the critical final store across both HWDGE
                        # rings so its two halves move concurrently.
                        half = NJ * P // 2
                        nc.sync.dma_start(
                            out[j, :, 0:half], ot[:, sl][:, 0:half]
                        )
                        nc.scalar.dma_start(
                            out[j, :, half:], ot[:, sl][:, half:]
                        )
                    else:
                        nc.sync.dma_start(out[j], ot[:, sl])

    nc.compile()
    return nc


def _get_nc(pattern, n_spk):
    key = (tuple(pattern), n_spk)
    if key not in _cache:
        _cache[key] = _build_nc(pattern, n_spk)
    return _cache[key]


def _prep_inputs(spikes, adjacency):
    import ml_dtypes

    flat = np.ascontiguousarray(np.asarray(spikes, dtype=np.float32).reshape(B, N))
    adj = np.asarray(adjacency, dtype=np.float32)

    # Live [ti, si] block map per core: block contributes to core i's
    # outputs iff adj[i*TSH + ti*128 : .. + 128, si*128 : (si+1)*128] has a
    # nonzero.
    bm = np.any(
        adj.reshape(NCORES, TI, W, SCHUNKS, P) != 0.0, axis=(2, 4)
    )  # [core, ti, si]

    # Align per-core block sets into one shared pattern via a per-core
    # source offset o_i (cores run one SPMD program). o_i = min(si - ti)
    # over live blocks aligns banded structures exactly.
    offs = np.zeros(NCORES, np.int64)
    pat = set()
    for i in range(NCORES):
        tis, sis = np.nonzero(bm[i])
        offs[i] = (sis - tis).min() if len(tis) else 0
        pat.update(zip(tis.tolist(), (sis - offs[i]).tolist()))
    for ti in range(TI):  # every ti needs >=1 block so PSUM gets initialized
        if not any(t == ti for t, _ in pat):
            pat.add((ti, 0))
    # si-major, ti-ascending stream order (enables merged matmuls over
    # consecutive ti sharing one stationary source chunk).
    pattern = sorted(pat, key=lambda x: (x[1], x[0]))
    n_spk = max(s for _, s in pattern) + 1

    # Stationary weights, indexed by absolute source chunk.
    flatT = np.ascontiguousarray(flat.T).astype(np.float16)  # [N, B]
    spk_full = flatT.reshape(SCHUNKS, P, B)

    n_blocks = len(pattern)
    g_sizes = _group_sizes(pattern)

    pat_ti = np.array([t for t, _ in pattern])
    pat_si_rel = np.array([s for _, s in pattern])
    in_maps = []
    scales = []
    for i in range(NCORES):
        o = int(offs[i])
        # Vectorized block gather: adj[t, s] viewed as [ti, tj, si, sj],
        # transposed per block to [sj, tj], quantized to fp8e3 (e3m4) with
        # a per-core scale that puts the max magnitude at ~FP8_MAX.
        a4 = adj[i * TSH : (i + 1) * TSH, :].reshape(TI, W, SCHUNKS, P)
        pat_si = pat_si_rel + o
        valid = (pat_si >= 0) & (pat_si < SCHUNKS)
        b32 = np.zeros((n_blocks, P, P), np.float32)  # [k, sj, tj]
        b32[valid] = a4[pat_ti[valid], :, pat_si[valid], :].transpose(0, 2, 1)
        amax = float(np.abs(b32).max())
        scale = FP8_MAX / amax if amax > 0 else 1.0
        b8 = (b32 * scale).astype(ml_dtypes.float8_e3m4)

        # Pack blocks partition-major per DMA group.
        parts = []
        k0 = 0
        for gsz in g_sizes:
            parts.append(
                np.ascontiguousarray(
                    b8[k0 : k0 + gsz].transpose(1, 0, 2)
                ).ravel()
            )
            k0 += gsz
        ablk = np.concatenate(parts)

        spkw = np.zeros((n_spk, P, B), np.float16)
        s_lo = max(0, -o)
        s_hi = min(n_spk, SCHUNKS - o)
        if s_hi > s_lo:
            spkw[s_lo:s_hi] = spk_full[o + s_lo : o + s_hi]
        spkw = np.ascontiguousarray(spkw.transpose(1, 0, 2)).reshape(P, n_spk * B)
        in_maps.append({"ablk": ablk, "spk": spkw})
        scales.append(scale)
    return pattern, n_spk, in_maps, scales


def _run(pattern, n_spk, in_maps, **kwargs):
    from concourse.bass_utils import run_bass_kernel_spmd

    return run_bass_kernel_spmd(
        _get_nc(pattern, n_spk), in_maps, core_ids=list(range(NCORES)), **kwargs
    )


def kernel(spikes, adjacency):
    pattern, n_spk, in_maps, scales = _prep_inputs(spikes, adjacency)
    res = _run(pattern, n_spk, in_maps)
    outs = [
        r["o"].astype(np.float32).transpose(1, 0, 2).reshape(B, TSH)
        * np.float32(1.0 / s)
        for r, s in zip(res.results, scales)
    ]
    full = np.concatenate(outs, axis=1)  # [B, N]
    return np.ascontiguousarray(full.reshape(B, H, W), dtype=np.float32)


# revision 44
# speedup vs baseline: 1.1134x; 1.1134x over previous
# Trainium2 Bass kernel for nn_AxonalConnections (gnn_message_passing).
#
# Computes out[B, H, W] = (spikes.reshape(B, N) @ adjacency.T).reshape(B, H, W)
# with B=16, H=W=128, N=16384 on 8 NeuronCores.
#
# Strategy (pure tensor parallelism, no collectives):
#   - Shard adjacency row-wise (target dim) across 8 cores: core i owns
#     target columns [i*2048, (i+1)*2048) of the output.
#   - The kernel is HBM/DMA-bandwidth bound, so minimize shipped bytes:
#     * input-adaptive block pruning: the host scans the adjacency at
#       [128 x 128] block granularity (source grid-row si x target grid-row
#       ti) and ships only blocks that contain nonzeros. For the conv-
#       structured adjacency this is ~112 of 2048 blocks per core (1.75 MiB
#       vs 64 MiB fp8); for dense inputs every block ships and the kernel
#       stays exact up to quantization. Per-core block sets are aligned by
#       a per-core source offset into one shared pattern so all 8 cores run
#       the same NEFF.
#     * fp8e3 (e3m4) adjacency stream with a per-core scale chosen so the
#       largest magnitude lands at ~8.0: measured 8.7e-3 relative output
#       error on the conv-structured data — 2.3x inside the 2e-2 accuracy
#       budget — at half the bytes of fp16. Spikes stay fp16 (stationary,
#       tiny). The host divides the scale back out of the output.
#   - Blocks stream si-major with merged matmuls over consecutive ti; each
#     PSUM bank finishes early and its PSUM->SBUF copy + output DMA overlap
#     the remaining matmuls. The final bank's store is split across both
#     HWDGE rings so its halves move concurrently. Output is bank-major in
#     DRAM so stores coalesce; the host de-interleaves.
#   - A PE warmup (24 x N=128 matmuls on a zeroed tile) spans the HAM cold
#     window while the first DMA groups arrive and bridges into the real
#     matmuls with no PE-idle gap, so they run at 2.4 GHz almost
#     immediately.
#   - DMA groups are aligned to si-run boundaries (a group edge never
#     splits a matmul segment). Head groups are small (4..16 blocks) so the
#     parade of per-group completion receipts starts the PE early; middle
#     groups of ~24 blocks amortize receipt overhead; a small tail group
#     keeps the after-last-byte matmul residue short.

import numpy as np

B = 16
H = 128
W = 128
N = H * W            # 16384 source == target size
NCORES = 8
TSH = N // NCORES    # 2048 target columns per core
TI = TSH // W        # 16 target grid-rows per core
P = 128              # SBUF partitions / contraction tile
SCHUNKS = N // P     # 128 source chunks (== source grid-rows)
BLK = P * P          # elements per block
FP8_MAX = 8.0        # target magnitude for the fp8e3 (e3m4) quantization
N_WARM = 24          # PE warmup matmuls (N=128 each; must bridge into the
                     # real matmuls with no PE-idle gap or HAM re-throttles)

_cache = {}


def _group_sizes(pattern):
    """DMA group sizes (blocks per dma_start), aligned to si-run boundaries.

    A matmul segment never spans two DMA groups, so aligning group edges to
    si-run edges avoids splitting segments (each split costs an extra
    LDWEIGHTS + matmul dispatch on the PE). Head groups are small so the
    parade of per-group completion receipts starts the PE early; a small
    tail group keeps the after-last-byte matmul residue short."""
    run_sizes = []
    for _, s in pattern:
        if run_sizes and s == run_sizes[-1][0]:
            run_sizes[-1][1] += 1
        else:
            run_sizes.append([s, 1])
    runs = [r[1] for r in run_sizes]
    targets = [4, 8, 12, 16] + [24] * 100
    sizes = []
    cur = 0
    ti_idx = 0
    total = sum(runs)
    done = 0
    for r in runs:
        cur += r
        done += r
        rem = total - done
        tgt = targets[min(ti_idx, len(targets) - 1)]
        # close the group when it reaches its target, but keep the final
        # group small (it sets the post-stream matmul residue).
        if cur >= tgt and (rem == 0 or rem >= 4):
            sizes.append(cur)
            cur = 0
            ti_idx += 1
    if cur:
        sizes.append(cur)
    return sizes


def _plan_segments(pattern, group_sizes):
    """Plan merged matmuls over the si-major block stream.

    pattern: list of (ti, si_rel), si-major then ti-ascending — the stream
    order. Blocks with consecutive ti, the same source chunk, the same PSUM
    bank, and the same DMA group merge into one matmul of N = 128*len.

    start=True is set ONLY on the first segment of each PSUM bank: on HW it
    clears has_written for the WHOLE bank, and the per-element has_written
    bit then makes every region's first write an overwrite and later writes
    accumulate.

    Returns segments: list of (k0, nblk, si_rel, ti0, start).
    """
    group_of = []
    for g, gsz in enumerate(group_sizes):
        group_of += [g] * gsz
    segments = []
    k = 0
    n = len(pattern)
    seen_banks = set()
    while k < n:
        ti0, s = pattern[k]
        ln = 1
        while (
            k + ln < n
            and pattern[k + ln] == (ti0 + ln, s)
            and (ti0 + ln) // 4 == ti0 // 4
            and group_of[k + ln] == group_of[k]
        ):
            ln += 1
        bank = ti0 // 4
        segments.append((k, ln, s, ti0, bank not in seen_banks))
        seen_banks.add(bank)
        k += ln
    return segments


def _build_nc(pattern, n_spk):
    """Build + compile the SPMD Bass program.

    pattern: list of (ti, si_rel) block coordinates in si-major stream
             order, identical for all cores. Every ti in [0, TI) appears.
    n_spk:   number of stationary source chunks shipped (max si_rel + 1).
    """
    import concourse.mybir as mybir
    import concourse.tile as tile
    from concourse import bacc

    n_blocks = len(pattern)
    g_sizes = _group_sizes(pattern)
    segs = _plan_segments(pattern, g_sizes)

    nc = bacc.Bacc(
        "TRN2",
        target_bir_lowering=False,
        debug=False,
        num_devices=NCORES,
        enable_partition_id=False,
    )
    # ablk: flat stream of gathered [128 x 128] fp8e3 adjacency blocks in
    # `pattern` order, packed per DMA group as [p, group_blocks*128]
    # (partition-major) so every descriptor moves one contiguous run per
    # partition.
    ablk = nc.dram_tensor(
        "ablk", [n_blocks * BLK], mybir.dt.float8e3, kind="ExternalInput"
    ).ap()
    # spk: stationary weights for the shipped source-chunk window, packed
    # [P, n_spk*B] fp16 where spk[p, k*B + b] = spikes[b, (o_i + k)*128 + p]
    # (o_i = per-core source offset; out-of-range chunks are zero).
    spk = nc.dram_tensor(
        "spk", [P, n_spk * B], mybir.dt.float16, kind="ExternalInput"
    ).ap()
    # Output is bank-major: o[j] is bank j's [B, 512] slab, contiguous in
    # DRAM so each store coalesces into large AXI bursts (a [B, TSH]-strided
    # store measured ~20 B/ns from descriptor overhead). fp16 halves the
    # drain copy (2x DVE mode) and the store bytes; the ~2e-4 rounding it
    # adds is negligible against the fp8 weight quantization. Host converts
    # back to fp32, rescales and de-interleaves.
    NJ = 4
    out = nc.dram_tensor(
        "o", [NJ, B, NJ * P], mybir.dt.float16, kind="ExternalOutput"
    ).ap()

    f32 = mybir.dt.float32
    f16 = mybir.dt.float16

    # Last stream index per PSUM bank (closes the accumulation group) and
    # the bank that closes last (its drain is the critical tail).
    last_k_bank = {}
    for k, (ti, _) in enumerate(pattern):
        last_k_bank[ti // NJ] = k
    final_bank = max(last_k_bank, key=last_k_bank.get)

    # Map stream index -> (group, local index).
    grp_of = []
    for g, gsz in enumerate(g_sizes):
        base = len(grp_of)
        grp_of += [(g, kk - base) for kk in range(base, base + gsz)]

    with tile.TileContext(nc) as tc:
        with (
            tc.tile_pool(name="adj", bufs=len(g_sizes)) as adj_pool,
            tc.tile_pool(name="spkp", bufs=1) as spk_pool,
            tc.tile_pool(name="warm", bufs=1) as warm_pool,
            tc.tile_pool(name="psum", bufs=1, space="PSUM") as psum_pool,
            tc.tile_pool(name="outp", bufs=1) as out_pool,
        ):
            ps = [
                psum_pool.tile([B, NJ * P], f32, name=f"ps{j}", tag=f"ps{j}")
                for j in range(NJ)
            ]

            # PE warmup: short N=128 matmuls on a zeroed tile into a scratch
            # PSUM bank, issued before any real data arrives. They keep the
            # PE busy across the HAM activity window so the real matmuls run
            # at 2.4 GHz, and they bridge into the real matmuls with no
            # PE-idle gap.
            dumt = warm_pool.tile([P, P], f16)
            nc.vector.memset(dumt[:], 0.0)
            psw = psum_pool.tile([B, P], f32, name="psw", tag="psw")
            for _ in range(N_WARM):
                nc.tensor.matmul(
                    psw[:, :],
                    dumt[:, 0:B],
                    dumt[:, :],
                    start=True,
                    stop=True,
                    skip_group_check=True,
                )

            # Stationary weights: the head chunks (what the first matmuls
            # need) go on the ACT ring immediately; the rest follows on the
            # same ring. Both stream concurrently with the first block
            # groups on the SP ring.
            spk_t = spk_pool.tile([P, n_spk * B], f16)
            n_head = min(4, n_spk)
            nc.scalar.dma_start(
                spk_t[:, : n_head * B], spk[:, : n_head * B]
            )
            if n_spk > n_head:
                nc.scalar.dma_start(
                    spk_t[:, n_head * B :], spk[:, n_head * B :]
                )

            ot = out_pool.tile([B, TSH], f16)

            # All input groups on the SP ring, in consumption order. The PE
            # consumes groups in order, so a second ring scrambles arrival
            # and starves it; and since the end of the stream is PE-bound
            # (matmuls trail the last input by ~2.5us), early tail arrival
            # via a second ring buys nothing while its head contention
            # delays the PE start — both measured as regressions.
            at_tiles = []
            off = 0
            for g, gsz in enumerate(g_sizes):
                at = adj_pool.tile(
                    [P, gsz * P], mybir.dt.float8e3, name=f"at{g}", tag="at"
                )
                nc.sync.dma_start(
                    at[:],
                    ablk[off : off + gsz * BLK].rearrange("(p t) -> p t", p=P),
                )
                off += gsz * BLK
                at_tiles.append(at)

            for k0, nblk, si_rel, ti0, start in segs:
                g, kl = grp_of[k0]
                j, c = divmod(ti0, NJ)
                pj = ps[j]
                nc.tensor.matmul(
                    pj[:, c * P : (c + nblk) * P],
                    spk_t[:, si_rel * B : (si_rel + 1) * B],
                    at_tiles[g][:, kl * P : (kl + nblk) * P],
                    start=start,
                    stop=(k0 + nblk - 1 == last_k_bank[j]),
                    skip_group_check=True,
                )
                if k0 + nblk - 1 == last_k_bank[j]:
                    # Bank fully accumulated: drain it while the remaining
                    # banks' matmuls keep streaming. Stores go on the SP
                    # ring: all input dma_start instructions precede them in
                    # the SP program, so a store's semaphore wait can never
                    # delay an input transfer. The final bank's copy is
                    # split across Vector + Scalar engines to halve the
                    # critical-path drain.
                    sl = slice(j * NJ * P, (j + 1) * NJ * P)
                    nc.vector.tensor_copy(ot[:, sl], pj[:, :])
                    if j == final_bank:
                        # Split the critical final store across both HWDGE
                        # rings so its two halves move concurrently.
                        half = NJ * P // 2
                        nc.sync.dma_start(
                            out[j, :, 0:half], ot[:, sl][:, 0:half]
                        )
                        nc.scalar.dma_start(
                            out[j, :, half:], ot[:, sl][:, half:]
                        )
                    else:
                        nc.sync.dma_start(out[j], ot[:, sl])

    nc.compile()
    return nc


def _get_nc(pattern, n_spk):
    key = (tuple(pattern), n_spk)
    if key not in _cache:
        _cache[key] = _build_nc(pattern, n_spk)
    return _cache[key]


def _prep_inputs(spikes, adjacency):
    import ml_dtypes

    flat = np.ascontiguousarray(np.asarray(spikes, dtype=np.float32).reshape(B, N))
    adj = np.asarray(adjacency, dtype=np.float32)

    # Live [ti, si] block map per core: block contributes to core i's
    # outputs iff adj[i*TSH + ti*128 : .. + 128, si*128 : (si+1)*128] has a
    # nonzero.
    bm = np.any(
        adj.reshape(NCORES, TI, W, SCHUNKS, P) != 0.0, axis=(2, 4)
    )  # [core, ti, si]

    # Align per-core block sets into one shared pattern via a per-core
    # source offset o_i (cores run one SPMD program). o_i = min(si - ti)
    # over live blocks aligns banded structures exactly.
    offs = np.zeros(NCORES, np.int64)
    pat = set()
    for i in range(NCORES):
        tis, sis = np.nonzero(bm[i])
        offs[i] = (sis - tis).min() if len(tis) else 0
        pat.update(zip(tis.tolist(), (sis - offs[i]).tolist()))
    for ti in range(TI):  # every ti needs >=1 block so PSUM gets initialized
        if not any(t == ti for t, _ in pat):
            pat.add((ti, 0))
    # si-major, ti-ascending stream order (enables merged matmuls over
    # consecutive ti sharing one stationary source chunk).
    pattern = sorted(pat, key=lambda x: (x[1], x[0]))
    n_spk = max(s for _, s in pattern) + 1

    # Stationary weights, indexed by absolute source chunk.
    flatT = np.ascontiguousarray(flat.T).astype(np.float16)  # [N, B]
    spk_full = flatT.reshape(SCHUNKS, P, B)

    n_blocks = len(pattern)
    g_sizes = _group_sizes(pattern)

    pat_ti = np.array([t for t, _ in pattern])
    pat_si_rel = np.array([s for _, s in pattern])
    in_maps = []
    scales = []
    for i in range(NCORES):
        o = int(offs[i])
        # Vectorized block gather: adj[t, s] viewed as [ti, tj, si, sj],
        # transposed per block to [sj, tj], quantized to fp8e3 (e3m4) with
        # a per-core scale that puts the max magnitude at ~FP8_MAX.
        a4 = adj[i * TSH : (i + 1) * TSH, :].reshape(TI, W, SCHUNKS, P)
        pat_si = pat_si_rel + o
        valid = (pat_si >= 0) & (pat_si < SCHUNKS)
        b32 = np.zeros((n_blocks, P, P), np.float32)  # [k, sj, tj]
        b32[valid] = a4[pat_ti[valid], :, pat_si[valid], :].transpose(0, 2, 1)
        amax = float(np.abs(b32).max())
        scale = FP8_MAX / amax if amax > 0 else 1.0
        b8 = (b32 * scale).astype(ml_dtypes.float8_e3m4)

        # Pack blocks partition-major per DMA group.
        parts = []
        k0 = 0
        for gsz in g_sizes:
            parts.append(
                np.ascontiguousarray(
                    b8[k0 : k0 + gsz].transpose(1, 0, 2)
                ).ravel()
            )
            k0 += gsz
        ablk = np.concatenate(parts)

        spkw = np.zeros((n_spk, P, B), np.float16)
        s_lo = max(0, -o)
        s_hi = min(n_spk, SCHUNKS - o)
        if s_hi > s_lo:
            spkw[s_lo:s_hi] = spk_full[o + s_lo : o + s_hi]
        spkw = np.ascontiguousarray(spkw.transpose(1, 0, 2)).reshape(P, n_spk * B)
        in_maps.append({"ablk": ablk, "spk": spkw})
        scales.append(scale)
    return pattern, n_spk, in_maps, scales


def _run(pattern, n_spk, in_maps, **kwargs):
    from concourse.bass_utils import run_bass_kernel_spmd

    return run_bass_kernel_spmd(
        _get_nc(pattern, n_spk), in_maps, core_ids=list(range(NCORES)), **kwargs
    )


def kernel(spikes, adjacency):
    pattern, n_spk, in_maps, scales = _prep_inputs(spikes, adjacency)
    res = _run(pattern, n_spk, in_maps)
    outs = [
        r["o"].astype(np.float32).transpose(1, 0, 2).reshape(B, TSH)
        * np.float32(1.0 / s)
        for r, s in zip(res.results, scales)
    ]
    full = np.concatenate(outs, axis=1)  # [B, N]
    return np.ascontiguousarray(full.reshape(B, H, W), dtype=np.float32)


# revision 45
# speedup vs baseline: 1.1245x; 1.0099x over previous
# Trainium2 Bass kernel for nn_AxonalConnections (gnn_message_passing).
#
# Computes out[B, H, W] = (spikes.reshape(B, N) @ adjacency.T).reshape(B, H, W)
# with B=16, H=W=128, N=16384 on 8 NeuronCores.
#
# Strategy (pure tensor parallelism, no collectives):
#   - Shard adjacency row-wise (target dim) across 8 cores: core i owns
#     target columns [i*2048, (i+1)*2048) of the output.
#   - The kernel is HBM/DMA-bandwidth bound, so minimize shipped bytes:
#     * input-adaptive block pruning: the host scans the adjacency at
#       [128 x 128] block granularity (source grid-row si x target grid-row
#       ti) and ships only blocks that contain nonzeros. For the conv-
#       structured adjacency this is ~112 of 2048 blocks per core (1.75 MiB
#       vs 64 MiB fp8); for dense inputs every block ships and the kernel
#       stays exact up to quantization. Per-core block sets are aligned by
#       a per-core source offset into one shared pattern so all 8 cores run
#       the same NEFF.
#     * fp8e3 (e3m4) adjacency stream with a per-core scale chosen so the
#       largest magnitude lands at ~8.0: measured 8.7e-3 relative output
#       error on the conv-structured data — 2.3x inside the 2e-2 accuracy
#       budget — at half the bytes of fp16. Spikes stay fp16 (stationary,
#       tiny). The host divides the scale back out of the output.
#   - Blocks stream si-major with merged matmuls over consecutive ti; each
#     PSUM bank finishes early and its PSUM->SBUF copy + output DMA overlap
#     the remaining matmuls. The final bank's store is split across both
#     HWDGE rings so its halves move concurrently. Output is bank-major in
#     DRAM so stores coalesce; the host de-interleaves.
#   - A PE warmup (24 x N=128 matmuls on a zeroed tile) spans the HAM cold
#     window while the first DMA groups arrive and bridges into the real
#     matmuls with no PE-idle gap, so they run at 2.4 GHz almost
#     immediately.
#   - DMA groups are aligned to si-run boundaries (a group edge never
#     splits a matmul segment). Head groups are small (4..16 blocks) so the
#     parade of per-group completion receipts starts the PE early; middle
#     groups of ~24 blocks amortize receipt overhead; a small tail group
#     keeps the after-last-byte matmul residue short.

import numpy as np

B = 16
H = 128
W = 128
N = H * W            # 16384 source == target size
NCORES = 8
TSH = N // NCORES    # 2048 target columns per core
TI = TSH // W        # 16 target grid-rows per core
P = 128              # SBUF partitions / contraction tile
SCHUNKS = N // P     # 128 source chunks (== source grid-rows)
BLK = P * P          # elements per block
FP8_MAX = 8.0        # target magnitude for the fp8e3 (e3m4) quantization
N_WARM = 24          # PE warmup matmuls (N=128 each; must bridge into the
                     # real matmuls with no PE-idle gap or HAM re-throttles)

_cache = {}


def _group_sizes(pattern):
    """DMA group sizes (blocks per dma_start), aligned to si-run boundaries.

    A matmul segment never spans two DMA groups, so aligning group edges to
    si-run edges avoids splitting segments (each split costs an extra
    LDWEIGHTS + matmul dispatch on the PE). Head groups are small so the
    parade of per-group completion receipts starts the PE early; a small
    tail group keeps the after-last-byte matmul residue short."""
    run_sizes = []
    for _, s in pattern:
        if run_sizes and s == run_sizes[-1][0]:
            run_sizes[-1][1] += 1
        else:
            run_sizes.append([s, 1])
    runs = [r[1] for r in run_sizes]
    targets = [4, 8, 12, 16] + [20] * 100
    sizes = []
    cur = 0
    ti_idx = 0
    total = sum(runs)
    done = 0
    for r in runs:
        cur += r
        done += r
        rem = total - done
        tgt = targets[min(ti_idx, len(targets) - 1)]
        # close the group when it reaches its target, but keep the final
        # group small (it sets the post-stream matmul residue).
        if cur >= tgt and (rem == 0 or rem >= 4):
            sizes.append(cur)
            cur = 0
            ti_idx += 1
    if cur:
        sizes.append(cur)
    return sizes


def _plan_segments(pattern, group_sizes):
    """Plan merged matmuls over the si-major block stream.

    pattern: list of (ti, si_rel), si-major then ti-ascending — the stream
    order. Blocks with consecutive ti, the same source chunk, the same PSUM
    bank, and the same DMA group merge into one matmul of N = 128*len.

    start=True is set ONLY on the first segment of each PSUM bank: on HW it
    clears has_written for the WHOLE bank, and the per-element has_written
    bit then makes every region's first write an overwrite and later writes
    accumulate.

    Returns segments: list of (k0, nblk, si_rel, ti0, start).
    """
    group_of = []
    for g, gsz in enumerate(group_sizes):
        group_of += [g] * gsz
    segments = []
    k = 0
    n = len(pattern)
    seen_banks = set()
    while k < n:
        ti0, s = pattern[k]
        ln = 1
        while (
            k + ln < n
            and pattern[k + ln] == (ti0 + ln, s)
            and (ti0 + ln) // 4 == ti0 // 4
            and group_of[k + ln] == group_of[k]
        ):
            ln += 1
        bank = ti0 // 4
        segments.append((k, ln, s, ti0, bank not in seen_banks))
        seen_banks.add(bank)
        k += ln
    return segments


def _build_nc(pattern, n_spk):
    """Build + compile the SPMD Bass program.

    pattern: list of (ti, si_rel) block coordinates in si-major stream
             order, identical for all cores. Every ti in [0, TI) appears.
    n_spk:   number of stationary source chunks shipped (max si_rel + 1).
    """
    import concourse.mybir as mybir
    import concourse.tile as tile
    from concourse import bacc

    n_blocks = len(pattern)
    g_sizes = _group_sizes(pattern)
    segs = _plan_segments(pattern, g_sizes)

    nc = bacc.Bacc(
        "TRN2",
        target_bir_lowering=False,
        debug=False,
        num_devices=NCORES,
        enable_partition_id=False,
    )
    # ablk: flat stream of gathered [128 x 128] fp8e3 adjacency blocks in
    # `pattern` order, packed per DMA group as [p, group_blocks*128]
    # (partition-major) so every descriptor moves one contiguous run per
    # partition.
    ablk = nc.dram_tensor(
        "ablk", [n_blocks * BLK], mybir.dt.float8e3, kind="ExternalInput"
    ).ap()
    # spk: stationary weights for the shipped source-chunk window, packed
    # [P, n_spk*B] fp16 where spk[p, k*B + b] = spikes[b, (o_i + k)*128 + p]
    # (o_i = per-core source offset; out-of-range chunks are zero).
    spk = nc.dram_tensor(
        "spk", [P, n_spk * B], mybir.dt.float16, kind="ExternalInput"
    ).ap()
    # Output is bank-major: o[j] is bank j's [B, 512] slab, contiguous in
    # DRAM so each store coalesces into large AXI bursts (a [B, TSH]-strided
    # store measured ~20 B/ns from descriptor overhead). fp16 halves the
    # drain copy (2x DVE mode) and the store bytes; the ~2e-4 rounding it
    # adds is negligible against the fp8 weight quantization. Host converts
    # back to fp32, rescales and de-interleaves.
    NJ = 4
    out = nc.dram_tensor(
        "o", [NJ, B, NJ * P], mybir.dt.float16, kind="ExternalOutput"
    ).ap()

    f32 = mybir.dt.float32
    f16 = mybir.dt.float16

    # Last stream index per PSUM bank (closes the accumulation group) and
    # the bank that closes last (its drain is the critical tail).
    last_k_bank = {}
    for k, (ti, _) in enumerate(pattern):
        last_k_bank[ti // NJ] = k
    final_bank = max(last_k_bank, key=last_k_bank.get)

    # Map stream index -> (group, local index).
    grp_of = []
    for g, gsz in enumerate(g_sizes):
        base = len(grp_of)
        grp_of += [(g, kk - base) for kk in range(base, base + gsz)]

    with tile.TileContext(nc) as tc:
        with (
            tc.tile_pool(name="adj", bufs=len(g_sizes)) as adj_pool,
            tc.tile_pool(name="spkp", bufs=1) as spk_pool,
            tc.tile_pool(name="warm", bufs=1) as warm_pool,
            tc.tile_pool(name="psum", bufs=1, space="PSUM") as psum_pool,
            tc.tile_pool(name="outp", bufs=1) as out_pool,
        ):
            ps = [
                psum_pool.tile([B, NJ * P], f32, name=f"ps{j}", tag=f"ps{j}")
                for j in range(NJ)
            ]

            # PE warmup: short N=128 matmuls on a zeroed tile into a scratch
            # PSUM bank, issued before any real data arrives. They keep the
            # PE busy across the HAM activity window so the real matmuls run
            # at 2.4 GHz, and they bridge into the real matmuls with no
            # PE-idle gap.
            dumt = warm_pool.tile([P, P], f16)
            nc.vector.memset(dumt[:], 0.0)
            psw = psum_pool.tile([B, P], f32, name="psw", tag="psw")
            for _ in range(N_WARM):
                nc.tensor.matmul(
                    psw[:, :],
                    dumt[:, 0:B],
                    dumt[:, :],
                    start=True,
                    stop=True,
                    skip_group_check=True,
                )

            # Stationary weights: the head chunks (what the first matmuls
            # need) go on the ACT ring immediately; the rest follows on the
            # same ring. Both stream concurrently with the first block
            # groups on the SP ring.
            spk_t = spk_pool.tile([P, n_spk * B], f16)
            n_head = min(4, n_spk)
            nc.scalar.dma_start(
                spk_t[:, : n_head * B], spk[:, : n_head * B]
            )
            if n_spk > n_head:
                nc.scalar.dma_start(
                    spk_t[:, n_head * B :], spk[:, n_head * B :]
                )

            ot = out_pool.tile([B, TSH], f16)

            # All input groups on the SP ring, in consumption order. The PE
            # consumes groups in order, so a second ring scrambles arrival
            # and starves it; and since the end of the stream is PE-bound
            # (matmuls trail the last input by ~2.5us), early tail arrival
            # via a second ring buys nothing while its head contention
            # delays the PE start — both measured as regressions.
            at_tiles = []
            off = 0
            for g, gsz in enumerate(g_sizes):
                at = adj_pool.tile(
                    [P, gsz * P], mybir.dt.float8e3, name=f"at{g}", tag="at"
                )
                nc.sync.dma_start(
                    at[:],
                    ablk[off : off + gsz * BLK].rearrange("(p t) -> p t", p=P),
                )
                off += gsz * BLK
                at_tiles.append(at)

            for k0, nblk, si_rel, ti0, start in segs:
                g, kl = grp_of[k0]
                j, c = divmod(ti0, NJ)
                pj = ps[j]
                nc.tensor.matmul(
                    pj[:, c * P : (c + nblk) * P],
                    spk_t[:, si_rel * B : (si_rel + 1) * B],
                    at_tiles[g][:, kl * P : (kl + nblk) * P],
                    start=start,
                    stop=(k0 + nblk - 1 == last_k_bank[j]),
                    skip_group_check=True,
                )
                if k0 + nblk - 1 == last_k_bank[j]:
                    # Bank fully accumulated: drain it while the remaining
                    # banks' matmuls keep streaming. Stores go on the SP
                    # ring: all input dma_start instructions precede them in
                    # the SP program, so a store's semaphore wait can never
                    # delay an input transfer. The final bank's copy is
                    # split across Vector + Scalar engines to halve the
                    # critical-path drain.
                    sl = slice(j * NJ * P, (j + 1) * NJ * P)
                    nc.vector.tensor_copy(ot[:, sl], pj[:, :])
                    if j == final_bank:
                        # Split the critical final store across both HWDGE
                        # rings so its two halves move concurrently.
                        half = NJ * P // 2
                        nc.sync.dma_start(
                            out[j, :, 0:half], ot[:, sl][:, 0:half]
                        )
                        nc.scalar.dma_start(
                            out[j, :, half:], ot[:, sl][:, half:]
                        )
                    else:
                        nc.sync.dma_start(out[j], ot[:, sl])

    nc.compile()
    return nc


def _get_nc(pattern, n_spk):
    key = (tuple(pattern), n_spk)
    if key not in _cache:
        _cache[key] = _build_nc(pattern, n_spk)
    return _cache[key]


def _prep_inputs(spikes, adjacency):
    import ml_dtypes

    flat = np.ascontiguousarray(np.asarray(spikes, dtype=np.float32).reshape(B, N))
    adj = np.asarray(adjacency, dtype=np.float32)

    # Live [ti, si] block map per core: block contributes to core i's
    # outputs iff adj[i*TSH + ti*128 : .. + 128, si*128 : (si+1)*128] has a
    # nonzero.
    bm = np.any(
        adj.reshape(NCORES, TI, W, SCHUNKS, P) != 0.0, axis=(2, 4)
    )  # [core, ti, si]

    # Align per-core block sets into one shared pattern via a per-core
    # source offset o_i (cores run one SPMD program). o_i = min(si - ti)
    # over live blocks aligns banded structures exactly.
    offs = np.zeros(NCORES, np.int64)
    pat = set()
    for i in range(NCORES):
        tis, sis = np.nonzero(bm[i])
        offs[i] = (sis - tis).min() if len(tis) else 0
        pat.update(zip(tis.tolist(), (sis - offs[i]).tolist()))
    for ti in range(TI):  # every ti needs >=1 block so PSUM gets initialized
        if not any(t == ti for t, _ in pat):
            pat.add((ti, 0))
    # si-major, ti-ascending stream order (enables merged matmuls over
    # consecutive ti sharing one stationary source chunk).
    pattern = sorted(pat, key=lambda x: (x[1], x[0]))
    n_spk = max(s for _, s in pattern) + 1

    # Stationary weights, indexed by absolute source chunk.
    flatT = np.ascontiguousarray(flat.T).astype(np.float16)  # [N, B]
    spk_full = flatT.reshape(SCHUNKS, P, B)

    n_blocks = len(pattern)
    g_sizes = _group_sizes(pattern)

    pat_ti = np.array([t for t, _ in pattern])
    pat_si_rel = np.array([s for _, s in pattern])
    in_maps = []
    scales = []
    for i in range(NCORES):
        o = int(offs[i])
        # Vectorized block gather: adj[t, s] viewed as [ti, tj, si, sj],
        # transposed per block to [sj, tj], quantized to fp8e3 (e3m4) with
        # a per-core scale that puts the max magnitude at ~FP8_MAX.
        a4 = adj[i * TSH : (i + 1) * TSH, :].reshape(TI, W, SCHUNKS, P)
        pat_si = pat_si_rel + o
        valid = (pat_si >= 0) & (pat_si < SCHUNKS)
        b32 = np.zeros((n_blocks, P, P), np.float32)  # [k, sj, tj]
        b32[valid] = a4[pat_ti[valid], :, pat_si[valid], :].transpose(0, 2, 1)
        amax = float(np.abs(b32).max())
        scale = FP8_MAX / amax if amax > 0 else 1.0
        b8 = (b32 * scale).astype(ml_dtypes.float8_e3m4)

        # Pack blocks partition-major per DMA group.
        parts = []
        k0 = 0
        for gsz in g_sizes:
            parts.append(
                np.ascontiguousarray(
                    b8[k0 : k0 + gsz].transpose(1, 0, 2)
                ).ravel()
            )
            k0 += gsz
        ablk = np.concatenate(parts)

        spkw = np.zeros((n_spk, P, B), np.float16)
        s_lo = max(0, -o)
        s_hi = min(n_spk, SCHUNKS - o)
        if s_hi > s_lo:
            spkw[s_lo:s_hi] = spk_full[o + s_lo : o + s_hi]
        spkw = np.ascontiguousarray(spkw.transpose(1, 0, 2)).reshape(P, n_spk * B)
        in_maps.append({"ablk": ablk, "spk": spkw})
        scales.append(scale)
    return pattern, n_spk, in_maps, scales


def _run(pattern, n_spk, in_maps, **kwargs):
    from concourse.bass_utils import run_bass_kernel_spmd

    return run_bass_kernel_spmd(
        _get_nc(pattern, n_spk), in_maps, core_ids=list(range(NCORES)), **kwargs
    )


def kernel(spikes, adjacency):
    pattern, n_spk, in_maps, scales = _prep_inputs(spikes, adjacency)
    res = _run(pattern, n_spk, in_maps)
    outs = [
        r["o"].astype(np.float32).transpose(1, 0, 2).reshape(B, TSH)
        * np.float32(1.0 / s)
        for r, s in zip(res.results, scales)
    ]
    full = np.concatenate(outs, axis=1)  # [B, N]
    return np.ascontiguousarray(full.reshape(B, H, W), dtype=np.float32)
